# revision 1
# baseline (speedup 1.0000x reference)
"""CloudCastV2 shifted-window transformer block on 8 trn2 NeuronCores.

Data-parallel over batch: 64 images -> 8 per core. Each core runs the full
block (LN1 -> shifted-window MHA -> gated residual -> LN2 -> MLP -> residual)
on its 8 images. The (-4,-4) roll + 8x8 window partition is folded into the
input/output DMA access patterns, so on chip everything lives in
"window-ordered" token space (8192 tokens x 512 ch per core).

Layouts on chip (per 256-token chunk = 4 windows):
  natural:    [128 tokens (partitions), C free]  - LN stats, skip/gate adds
  transposed: [C (partitions, 4 tiles), tokens]  - all dense matmuls (fp32r)
  attention:  qT/kT premasked bf16; per (window-pair, head) 128x128 qk^T with
              block-diag bias (-1e30 off-diag) -> exp -> ones-matmul denom ->
              reciprocal -> normalize -> AV matmul gives out^T directly.
"""

import numpy as np
import ml_dtypes

WS, SHIFT, HEADS, DIM, HRES, WRES = 8, 4, 8, 512, 32, 32
N = WS * WS            # 64 tokens / window
NH = HEADS
D = DIM // NH          # 64
B_TOTAL, NCORES = 64, 8
B_LOC = B_TOTAL // NCORES          # 8 images / core
TOK_IMG = HRES * WRES              # 1024
CHUNK = 256                        # tokens per chunk (4 windows)
NCHUNK = B_LOC * TOK_IMG // CHUNK  # 32
TT_CH = CHUNK // 128               # 128-token tiles per chunk (2)
WP_CH = TT_CH                      # window-pairs per chunk (2)
SCALE = float(D) ** -0.5
NEG = -1.0e30

F32 = None  # filled after mybir import
_prog_cache = {}


def _rel_index(ws):
    coords = np.arange(ws)
    grid = np.stack(np.meshgrid(coords, coords, indexing="ij"))
    flat = grid.reshape(2, -1)
    rel = flat[:, :, None] - flat[:, None, :]
    rel[0] += ws - 1
    rel[1] += ws - 1
    return rel[0] * (2 * ws - 1) + rel[1]


def _shift_mask(ws, shift):
    base = np.zeros((ws, ws), dtype=bool)
    base[ws - shift:, :] = True
    base[:, ws - shift:] = True
    return base.reshape(-1)


def _win_pieces(w):
    """DMA pieces for window w (0..15): list of (p0, np_, h0, q0, nq, w0).

    Window w = (wi, wj). Token (i, j) -> partition 8*i+j, source
    h=(8*wi+i+4)%32, w=(8*wj+j+4)%32. Returns pieces splitting the wrap.
    """
    wi, wj = w // 4, w % 4
    ih = [(0, 8, 8 * wi + 4)] if wi < 3 else [(0, 4, 28), (4, 4, 0)]
    jw = [(0, 8, 8 * wj + 4)] if wj < 3 else [(0, 4, 28), (4, 4, 0)]
    out = []
    for (i0, ni, h0) in ih:
        for (j0, nj, w0) in jw:
            out.append((i0, ni, h0, j0, nj, w0))
    return out


def _build_program():
    import concourse.bass as bass
    from concourse import bacc
    import concourse.mybir as mybir
    import concourse.tile as tile
    from concourse.masks import make_identity

    dt = mybir.dt
    f32, f32r, bf16 = dt.float32, dt.float32r, dt.bfloat16
    AF = mybir.ActivationFunctionType
    OP = mybir.AluOpType

    nc = bacc.Bacc("TRN2", target_bir_lowering=False, debug=True)
    x_d = nc.declare_dram_parameter("x", [B_LOC, TOK_IMG, DIM], f32, isOutput=False)
    y_d = nc.declare_dram_parameter("y", [B_LOC, TOK_IMG, DIM], f32, isOutput=True)
    wqT_d = nc.declare_dram_parameter("wqT", [DIM, DIM], f32, isOutput=False)
    wkT_d = nc.declare_dram_parameter("wkT", [DIM, DIM], f32, isOutput=False)
    wvT_d = nc.declare_dram_parameter("wvT", [DIM, DIM], f32, isOutput=False)
    wpT_d = nc.declare_dram_parameter("wpT", [DIM, DIM], f32, isOutput=False)
    w1T_d = nc.declare_dram_parameter("w1T", [DIM, 4 * DIM], f32, isOutput=False)
    w2T_d = nc.declare_dram_parameter("w2T", [4 * DIM, DIM], bf16, isOutput=False)
    bq_d = nc.declare_dram_parameter("bq", [DIM], f32, isOutput=False)
    bk_d = nc.declare_dram_parameter("bk", [DIM], f32, isOutput=False)
    bv_d = nc.declare_dram_parameter("bv", [DIM], f32, isOutput=False)
    bp_d = nc.declare_dram_parameter("bp", [DIM], f32, isOutput=False)
    b1_d = nc.declare_dram_parameter("b1", [4 * DIM], f32, isOutput=False)
    b2_d = nc.declare_dram_parameter("b2", [DIM], f32, isOutput=False)
    g1_d = nc.declare_dram_parameter("g1", [DIM], f32, isOutput=False)
    bl1_d = nc.declare_dram_parameter("bl1", [DIM], f32, isOutput=False)
    g2_d = nc.declare_dram_parameter("g2", [DIM], f32, isOutput=False)
    bl2_d = nc.declare_dram_parameter("bl2", [DIM], f32, isOutput=False)
    biasT_d = nc.declare_dram_parameter("biasT", [NH, 128, 128], f32, isOutput=False)
    qm_d = nc.declare_dram_parameter("qm", [128, CHUNK], f32, isOutput=False)
    sgw_d = nc.declare_dram_parameter("sgw", [8, 128], f32, isOutput=False)

    from contextlib import ExitStack

    with tile.TileContext(nc) as tc:
        with ExitStack() as es:
            P = lambda *a, **kw: es.enter_context(tc.tile_pool(*a, **kw))
            wts = P(name="wts", bufs=1)
            cst = P(name="cst", bufs=1)
            lnp = P(name="ln", bufs=4)
            xrp = P(name="xr", bufs=2)
            xcp = P(name="xc", bufs=1)
            xnTp = P(name="xnT", bufs=2)
            qkvp = P(name="qkv", bufs=2)
            ptp = P(name="pt", bufs=2)
            t3p = P(name="t3", bufs=2)
            rcp = P(name="rc", bufs=2)
            pnp = P(name="pn", bufs=4)
            aoTp = P(name="aoT", bufs=2)
            x2Tp = P(name="x2T", bufs=2)
            x3p = P(name="x3", bufs=2)
            xn2Tp = P(name="xn2T", bufs=2)
            h1Tp = P(name="h1T", bufs=1)
            h2Tp = P(name="h2T", bufs=2)
            yop = P(name="yo", bufs=1)
            psmm = P(name="psmm", bufs=3, space="PSUM")
            pssm = P(name="pssm", bufs=1, space="PSUM")
            psdn = P(name="psdn", bufs=1, space="PSUM")
            # ---- resident weights & constants ----
            WQ = [wts.tile([128, DIM], f32r, name=f"wq{i}") for i in range(4)]
            WK = [wts.tile([128, DIM], f32r, name=f"wk{i}") for i in range(4)]
            WV = [wts.tile([128, DIM], f32r, name=f"wv{i}") for i in range(4)]
            WP = [wts.tile([128, DIM], f32r, name=f"wp{i}") for i in range(4)]
            W1 = [wts.tile([128, 4 * DIM], f32r, name=f"w1{i}") for i in range(4)]
            for i in range(4):
                nc.gpsimd.dma_start(out=WQ[i], in_=wqT_d[128 * i:128 * (i + 1), :].bitcast(f32r))
                nc.gpsimd.dma_start(out=WK[i], in_=wkT_d[128 * i:128 * (i + 1), :].bitcast(f32r))
                nc.gpsimd.dma_start(out=WV[i], in_=wvT_d[128 * i:128 * (i + 1), :].bitcast(f32r))
                nc.gpsimd.dma_start(out=WP[i], in_=wpT_d[128 * i:128 * (i + 1), :].bitcast(f32r))
                nc.gpsimd.dma_start(out=W1[i], in_=w1T_d[128 * i:128 * (i + 1), :].bitcast(f32r))
            W2b = [wts.tile([128, DIM], bf16, name=f"w2b{i}") for i in range(16)]
            for i in range(16):
                nc.gpsimd.dma_start(out=W2b[i], in_=w2T_d[128 * i:128 * (i + 1), :])

            BIAS = [cst.tile([128, 128], f32, name=f"bias{h}") for h in range(NH)]
            for h in range(NH):
                nc.gpsimd.dma_start(out=BIAS[h], in_=biasT_d[h])
            QM = cst.tile([128, CHUNK], f32, name="qm")
            nc.gpsimd.dma_start(out=QM, in_=qm_d[:, :])
            SG = cst.tile([128, 8], f32, name="sg")
            nc.gpsimd.dma_start(out=SG, in_=sgw_d[:, :].rearrange("t p -> p t"))
            IDT = cst.tile([128, 128], f32, name="idt")
            make_identity(nc, IDT)
            ONES = cst.tile([128, 128], bf16, name="ones")
            nc.vector.memset(ONES, 1.0)
            EPS = cst.tile([128, 1], f32, name="eps")
            nc.vector.memset(EPS, 1e-5)

            def vec_sb(dram, n, name):
                t = cst.tile([128, n], f32, name=name)
                nc.gpsimd.dma_start(out=t, in_=dram[:].rearrange("(t p) -> p t", p=128))
                return t

            BQ = vec_sb(bq_d, 4, "bq")
            BK = vec_sb(bk_d, 4, "bk")
            BV = vec_sb(bv_d, 4, "bv")
            BP = vec_sb(bp_d, 4, "bp")
            B1 = vec_sb(b1_d, 16, "b1")
            B2 = vec_sb(b2_d, 4, "b2")
            G1 = vec_sb(g1_d, 4, "g1")
            BL1 = vec_sb(bl1_d, 4, "bl1")
            G2 = vec_sb(g2_d, 4, "g2")
            BL2 = vec_sb(bl2_d, 4, "bl2")

            def r32(ap):
                return ap.bitcast(f32r)

            # One-time DVE "touch" of every DMA-loaded tile: converts all
            # weight/const readiness into vector-engine program order so no
            # downstream instruction needs more than 2 sync waits.
            scr = cst.tile([128, 2048], f32, name="scr")
            touch_list = (WQ + WK + WV + WP + W1 + W2b + BIAS
                          + [QM, SG, IDT, BQ, BK, BV, BP, B1, B2,
                             G1, BL1, G2, BL2])
            for tt_ in touch_list:
                n_ = tt_.shape[-1] if len(tt_.shape) == 2 else 1
                src_ = tt_ if tt_.dtype in (f32, bf16) else tt_.bitcast(f32)
                if src_.dtype == bf16:
                    nc.vector.tensor_copy(out=scr.bitcast(bf16)[:, :n_], in_=src_)
                else:
                    nc.vector.tensor_copy(out=scr[:, :n_], in_=src_)

            def win_dma(tile_, b, w, p0, store=False):
                """window w of image b <-> tile partitions [p0:p0+64). One DMA
                per 128-token tile is issued by the caller via p0==0 path."""
                if p0 != 0:
                    return  # both windows handled in one DMA at p0 == 0
                t0 = 64 * w
                if store:
                    nc.gpsimd.dma_start(out=y_d[b, t0:t0 + 128, :], in_=tile_)
                else:
                    nc.gpsimd.dma_start(out=tile_, in_=x_d[b, t0:t0 + 128, :])

            def layer_norm(xin, tag):
                """per-token stats of xin [128, DIM] -> (x-m)*rstd (no g/b)."""
                st = lnp.tile([128, 6], f32, tag=f"st{tag}", name=f"st{tag}")
                nc.vector.bn_stats(out=st, in_=xin)
                mv = lnp.tile([128, 2], f32, tag=f"mv{tag}", name=f"mv{tag}")
                nc.vector.bn_aggr(out=mv, in_=st)
                sd = lnp.tile([128, 1], f32, tag=f"sd{tag}", name=f"sd{tag}")
                nc.scalar.activation(out=sd, in_=mv[:, 1:2], func=AF.Sqrt, bias=EPS)
                rs = lnp.tile([128, 1], f32, tag=f"rs{tag}", name=f"rs{tag}")
                nc.vector.reciprocal(out=rs, in_=sd)
                xc = xcp.tile([128, DIM], f32, tag=f"xc{tag}", name=f"xc{tag}")
                nc.vector.tensor_scalar(out=xc, in0=xin, scalar1=mv[:, 0:1],
                                        scalar2=rs, op0=OP.subtract, op1=OP.mult)
                return xc

            for ch in range(NCHUNK):
                b, qt = ch // 4, ch % 4
                wbase = 4 * qt

                # ---- load (window-ordered) + LN1 + transpose -> xnT ----
                xr = [xrp.tile([128, DIM], f32, tag=f"xr{t}", name=f"xr{t}") for t in range(TT_CH)]
                for t in range(TT_CH):
                    for k in range(2):
                        win_dma(xr[t], b, wbase + 2 * t + k, 64 * k)
                xnT = [xnTp.tile([128, CHUNK], f32r, tag=f"xnT{c}", name=f"xnT{c}") for c in range(4)]
                for t in range(TT_CH):
                    xc = layer_norm(xr[t], "1")
                    for c in range(4):
                        tp = pssm.tile([128, 128], f32, tag="tp", name="tp")
                        nc.tensor.transpose(tp, xc[:, 128 * c:128 * (c + 1)], IDT)
                        nc.vector.tensor_scalar(
                            out=xnT[c][:, 128 * t:128 * (t + 1)], in0=tp,
                            scalar1=G1[:, c:c + 1], scalar2=BL1[:, c:c + 1],
                            op0=OP.mult, op1=OP.add)

                # ---- QKV ----
                qT = [qkvp.tile([128, CHUNK], bf16, tag=f"qT{c}", name=f"qT{c}") for c in range(4)]
                kT = [qkvp.tile([128, CHUNK], bf16, tag=f"kT{c}", name=f"kT{c}") for c in range(4)]
                vN = [qkvp.tile([128, DIM], bf16, tag=f"vN{t}", name=f"vN{t}") for t in range(TT_CH)]
                for c in range(4):
                    ps = psmm.tile([128, CHUNK], f32, tag="mm", name="mm")
                    for ci in range(4):
                        nc.tensor.matmul(ps, WQ[ci][:, 128 * c:128 * (c + 1)],
                                         xnT[ci], start=(ci == 0), stop=(ci == 3))
                    tq = t3p.tile([128, CHUNK], f32, tag="tq", name="tq")
                    nc.vector.tensor_scalar(out=tq, in0=ps, scalar1=BQ[:, c:c + 1],
                                            scalar2=None, op0=OP.add)
                    nc.vector.tensor_mul(out=qT[c], in0=tq, in1=QM)
                    ps2 = psmm.tile([128, CHUNK], f32, tag="mm", name="mm")
                    for ci in range(4):
                        nc.tensor.matmul(ps2, WK[ci][:, 128 * c:128 * (c + 1)],
                                         xnT[ci], start=(ci == 0), stop=(ci == 3))
                    nc.scalar.activation(out=kT[c], in_=ps2, func=AF.Identity,
                                         bias=BK[:, c:c + 1])
                for t in range(TT_CH):
                    ps = psmm.tile([128, DIM], f32, tag="mm", name="mm")
                    for ci in range(4):
                        nc.tensor.matmul(ps, xnT[ci][:, 128 * t:128 * (t + 1)],
                                         WV[ci], start=(ci == 0), stop=(ci == 3))
                    nc.scalar.activation(out=vN[t], in_=ps, func=AF.Copy)

                # ---- attention ----
                aoT = [aoTp.tile([128, CHUNK], f32r, tag=f"aoT{c}", name=f"aoT{c}") for c in range(4)]
                for wp in range(WP_CH):
                    PT = ptp.tile([128, NH * 128], bf16, tag="pt", name="pt")
                    for h in range(NH):
                        cth, ro = h // 2, 64 * (h % 2)
                        sl = slice(128 * wp, 128 * (wp + 1))
                        qk = pssm.tile([128, 128], f32, tag="qk", name="qk")
                        nc.tensor.matmul(qk, kT[cth][ro:ro + 64, sl],
                                         qT[cth][ro:ro + 64, sl], start=True, stop=True)
                        t3 = t3p.tile([128, 128], f32, tag="t3", name="t3")
                        nc.vector.scalar_tensor_tensor(
                            out=t3, in0=qk, scalar=SCALE, in1=BIAS[h],
                            op0=OP.mult, op1=OP.add)
                        nc.scalar.activation(out=PT[:, 128 * h:128 * (h + 1)],
                                             in_=t3, func=AF.Exp)
                    rc = []
                    for g in range(2):
                        dn = psdn.tile([128, 512], f32, tag="dn", name="dn")
                        nc.tensor.matmul(dn, ONES, PT[:, 512 * g:512 * (g + 1)],
                                         start=True, stop=True)
                        r = rcp.tile([128, 512], bf16, tag=f"rc{g}", name=f"rc{g}")
                        with nc.allow_low_precision(reason="attn weights bf16"):
                            nc.vector.reciprocal(out=r, in_=dn)
                        rc.append(r)
                    for h in range(NH):
                        cth, ro = h // 2, 64 * (h % 2)
                        rcb = rc[h // 4][:, 128 * (h % 4):128 * (h % 4 + 1)]
                        pn = pnp.tile([128, 128], bf16, tag="pn", name="pn")
                        nc.gpsimd.tensor_mul(out=pn, in0=PT[:, 128 * h:128 * (h + 1)],
                                             in1=rcb)
                        av = pssm.tile([128, 128], f32, tag="av", name="av")
                        nc.tensor.matmul(av[ro:ro + 64, :],
                                         vN[wp][:, 64 * h:64 * (h + 1)], pn,
                                         start=True, stop=True,
                                         tile_position=(0, ro))
                        nc.vector.tensor_scalar(
                            out=aoT[cth][ro:ro + 64, 128 * wp:128 * (wp + 1)],
                            in0=av[ro:ro + 64, :], scalar1=BV[ro:ro + 64, cth:cth + 1],
                            scalar2=None, op0=OP.add)

                # ---- proj + residual (in T) ----
                x2T = [x2Tp.tile([128, CHUNK], f32, tag=f"x2T{c}", name=f"x2T{c}") for c in range(4)]
                for c in range(4):
                    ps = psmm.tile([128, CHUNK], f32, tag="mm", name="mm")
                    for ci in range(4):
                        nc.tensor.matmul(ps, WP[ci][:, 128 * c:128 * (c + 1)],
                                         aoT[ci], start=(ci == 0), stop=(ci == 3))
                    nc.vector.scalar_tensor_tensor(
                        out=x2T[c], in0=ps, scalar=BP[:, c:c + 1], in1=xnT[c],
                        op0=OP.add, op1=OP.add)

                # ---- back to natural: x3 = x2 + sig(gate)*x ----
                x3 = [x3p.tile([128, DIM], f32, tag=f"x3{t}", name=f"x3{t}") for t in range(TT_CH)]
                for c in range(4):
                    for t in range(TT_CH):
                        tp = pssm.tile([128, 128], f32, tag="tp", name="tp")
                        nc.tensor.transpose(tp, x2T[c][:, 128 * t:128 * (t + 1)], IDT)
                        col = 2 * qt + t
                        nc.vector.scalar_tensor_tensor(
                            out=x3[t][:, 128 * c:128 * (c + 1)],
                            in0=xr[t][:, 128 * c:128 * (c + 1)],
                            scalar=SG[:, col:col + 1], in1=tp,
                            op0=OP.mult, op1=OP.add)

                # ---- LN2 + transpose ----
                xn2T = [xn2Tp.tile([128, CHUNK], f32r, tag=f"xn2T{c}", name=f"xn2T{c}") for c in range(4)]
                for t in range(TT_CH):
                    xc2 = layer_norm(x3[t], "2")
                    for c in range(4):
                        tp = pssm.tile([128, 128], f32, tag="tp", name="tp")
                        nc.tensor.transpose(tp, xc2[:, 128 * c:128 * (c + 1)], IDT)
                        nc.vector.tensor_scalar(
                            out=xn2T[c][:, 128 * t:128 * (t + 1)], in0=tp,
                            scalar1=G2[:, c:c + 1], scalar2=BL2[:, c:c + 1],
                            op0=OP.mult, op1=OP.add)

                # ---- MLP ----
                h1 = [h1Tp.tile([128, CHUNK], bf16, tag=f"h1_{o}", name=f"h1_{o}") for o in range(16)]
                for o in range(16):
                    ps = psmm.tile([128, CHUNK], f32, tag="mm", name="mm")
                    for ci in range(4):
                        nc.tensor.matmul(ps, W1[ci][:, 128 * o:128 * (o + 1)],
                                         xn2T[ci], start=(ci == 0), stop=(ci == 3))
                    nc.scalar.activation(out=h1[o], in_=ps, func=AF.Gelu,
                                         bias=B1[:, o:o + 1])
                h2T = [h2Tp.tile([128, CHUNK], f32, tag=f"h2T{c}", name=f"h2T{c}") for c in range(4)]
                for c in range(4):
                    ps = psmm.tile([128, CHUNK], f32, tag="mm", name="mm")
                    for hi in range(16):
                        nc.tensor.matmul(ps, W2b[hi][:, 128 * c:128 * (c + 1)],
                                         h1[hi], start=(hi == 0), stop=(hi == 15))
                    nc.scalar.activation(out=h2T[c], in_=ps, func=AF.Identity,
                                         bias=B2[:, c:c + 1])

                # ---- final add + store ----
                for t in range(TT_CH):
                    yo = yop.tile([128, DIM], f32, tag=f"yo{t}", name=f"yo{t}")
                    for c in range(4):
                        tp = pssm.tile([128, 128], f32, tag="tp", name="tp")
                        nc.tensor.transpose(tp, h2T[c][:, 128 * t:128 * (t + 1)], IDT)
                        nc.vector.tensor_add(out=yo[:, 128 * c:128 * (c + 1)],
                                             in0=tp, in1=x3[t][:, 128 * c:128 * (c + 1)])
                    for k in range(2):
                        win_dma(yo, b, wbase + 2 * t + k, 64 * k, store=True)

    nc.compile()
    return nc


def _host_consts(rel_table):
    idx = _rel_index(WS).reshape(-1)
    bias = rel_table.reshape(-1, NH)[idx].reshape(N, NH, N)  # [n, h, m]
    qmask = _shift_mask(WS, SHIFT)                           # [64] True=masked
    keep = (~qmask).astype(np.float32)
    biasT = np.full((NH, 128, 128), NEG, np.float32)
    for h in range(NH):
        bT = bias[:, h, :].T * keep[None, :]                 # [m, n] masked cols->0
        biasT[h, :64, :64] = bT
        biasT[h, 64:, 64:] = bT
    qm = np.tile(keep, CHUNK // N)[None, :].repeat(128, 0).astype(np.float32)
    return biasT, qm


def _win_order_sigmoid_gate(gate):
    g = 1.0 / (1.0 + np.exp(-gate.reshape(HRES, WRES).astype(np.float64)))
    g = g.astype(np.float32)
    sg = np.zeros((16, 64), np.float32)
    for w in range(16):
        wi, wj = w // 4, w % 4
        for i in range(8):
            for j in range(8):
                sg[w, 8 * i + j] = g[(8 * wi + i + 4) % 32, (8 * wj + j + 4) % 32]
    return sg.reshape(8, 128)


_PERM = None


def _perm_idx():
    global _PERM
    if _PERM is None:
        p = np.zeros(1024, np.int64)
        for w in range(16):
            for (i0, ni, h0, j0, nj, w0) in _win_pieces(w):
                for a in range(ni):
                    for bb in range(nj):
                        p[64 * w + 8 * (i0 + a) + (j0 + bb)] = (h0 + a) * WRES + (w0 + bb)
        _PERM = p
    return _PERM


def kernel(**inputs):
    from concourse.bass_utils import run_bass_kernel_spmd

    x = np.asarray(inputs["x"], np.float32)           # (64,1,32,32,512)
    biasT, qm = _host_consts(np.asarray(inputs["rel_table"], np.float32))
    sgw = _win_order_sigmoid_gate(np.asarray(inputs["gate"], np.float32))
    common = {
        "wqT": np.ascontiguousarray(np.asarray(inputs["wq"], np.float32).T),
        "wkT": np.ascontiguousarray(np.asarray(inputs["wk"], np.float32).T),
        "wvT": np.ascontiguousarray(np.asarray(inputs["wv"], np.float32).T),
        "wpT": np.ascontiguousarray(np.asarray(inputs["wp"], np.float32).T),
        "w1T": np.ascontiguousarray(np.asarray(inputs["mlp_w1"], np.float32).T),
        "w2T": np.ascontiguousarray(np.asarray(inputs["mlp_w2"], np.float32).T).astype(ml_dtypes.bfloat16),
        "bq": np.asarray(inputs["bq"], np.float32),
        "bk": np.asarray(inputs["bk"], np.float32),
        "bv": np.asarray(inputs["bv"], np.float32),
        "bp": np.asarray(inputs["bp"], np.float32),
        "b1": np.asarray(inputs["mlp_b1"], np.float32),
        "b2": np.asarray(inputs["mlp_b2"], np.float32),
        "g1": np.asarray(inputs["ln1_g"], np.float32),
        "bl1": np.asarray(inputs["ln1_b"], np.float32),
        "g2": np.asarray(inputs["ln2_g"], np.float32),
        "bl2": np.asarray(inputs["ln2_b"], np.float32),
        "biasT": biasT, "qm": qm, "sgw": sgw,
    }
    if "prog" not in _prog_cache:
        _prog_cache["prog"] = _build_program()
    nc = _prog_cache["prog"]

    perm = _perm_idx()
    xw = x.reshape(B_TOTAL, TOK_IMG, DIM)[:, perm, :]   # window-ordered
    in_maps = []
    for c in range(NCORES):
        m = dict(common)
        m["x"] = np.ascontiguousarray(xw[c * B_LOC:(c + 1) * B_LOC])
        in_maps.append(m)
    res = run_bass_kernel_spmd(nc, in_maps, core_ids=list(range(NCORES)))
    yw = np.concatenate([res.results[c]["y"] for c in range(NCORES)], axis=0)
    out = np.empty((B_TOTAL, TOK_IMG, DIM), np.float32)
    out[:, perm, :] = yw
    return out.reshape(B_TOTAL, 1, HRES, WRES, DIM).astype(np.float32)



# revision 23
# speedup vs baseline: 2.5919x; 2.5919x over previous
"""CloudCastV2 shifted-window transformer block on 8 trn2 NeuronCores.

Data-parallel over batch: 64 images -> 8 per core. The (-4,-4) roll + 8x8
window partition is folded into host-side permutation; on chip everything is
window-ordered token space (8192 tokens x 512 ch per core), processed in 32
chunks of 256 tokens (4 windows / 2 window-pairs).

v2 design:
  - all dense matmuls (QKV / proj / MLP) run in fp8(e4m3) DoubleRow perf
    mode: weights are scaled x64 on host, the 1/64 is folded into the
    PSUM->SBUF copy scale. LN gammas/betas folded into weights/biases.
  - transposes are bf16 PE transposes (1 cycle/row); activations live in
    bf16, residual stream in f32.
  - attention: qk^T (bf16) + relative-position bias added via a second
    matmul (bias stationary, identity moving) -> ACT Exp -> ones-matmul
    denominator -> DVE reciprocal -> pn = PT*rc (bf16 2x) -> AV.
  - rstd = exp(-0.5*ln(var+eps)) so LN shares the Exp ACT table; only the
    MLP Gelu needs a second table. Chunks are processed in groups of 4
    with phase-grouped ACT usage (manually placed LoadActFuncSet + same-
    engine deps) -> 2 table swaps per 4 chunks; group g+1's loads+LN1+QKV
    are emitted before group g's MLP phase (software pipelining).
  - elementwise work balanced across DVE / ACT / Pool; x/y DMAs issued from
    the SP sequencer via HWDGE (Pool only issues the one-time weight loads).
"""

import numpy as np
import ml_dtypes

WS, SHIFT, HEADS, DIM, HRES, WRES = 8, 4, 8, 512, 32, 32
N = WS * WS
NH = HEADS
D = DIM // NH
B_TOTAL, NCORES = 64, 8
B_LOC = B_TOTAL // NCORES
TOK_IMG = HRES * WRES
CHUNK = 256
NCHUNK = B_LOC * TOK_IMG // CHUNK   # 32
SCALE = float(D) ** -0.5
NEG = -30000.0
WSC = 64.0                           # fp8 weight prescale
F8 = ml_dtypes.float8_e4m3

_prog_cache = {}


def _rel_index(ws):
    coords = np.arange(ws)
    grid = np.stack(np.meshgrid(coords, coords, indexing="ij"))
    flat = grid.reshape(2, -1)
    rel = flat[:, :, None] - flat[:, None, :]
    rel[0] += ws - 1
    rel[1] += ws - 1
    return rel[0] * (2 * ws - 1) + rel[1]


def _shift_mask(ws, shift):
    base = np.zeros((ws, ws), dtype=bool)
    base[ws - shift:, :] = True
    base[:, ws - shift:] = True
    return base.reshape(-1)


def _win_pieces(w):
    wi, wj = w // 4, w % 4
    ih = [(0, 8, 8 * wi + 4)] if wi < 3 else [(0, 4, 28), (4, 4, 0)]
    jw = [(0, 8, 8 * wj + 4)] if wj < 3 else [(0, 4, 28), (4, 4, 0)]
    out = []
    for (i0, ni, h0) in ih:
        for (j0, nj, w0) in jw:
            out.append((i0, ni, h0, j0, nj, w0))
    return out


_PERM = None


def _perm_idx():
    global _PERM
    if _PERM is None:
        p = np.zeros(1024, np.int64)
        for w in range(16):
            for (i0, ni, h0, j0, nj, w0) in _win_pieces(w):
                for a in range(ni):
                    for bb in range(nj):
                        p[64 * w + 8 * (i0 + a) + (j0 + bb)] = (h0 + a) * WRES + (w0 + bb)
        _PERM = p
    return _PERM


def _win_order_sigmoid_gate(gate):
    g = 1.0 / (1.0 + np.exp(-gate.reshape(HRES, WRES).astype(np.float64)))
    g = g.astype(np.float32)
    sg = np.zeros((16, 64), np.float32)
    for w in range(16):
        wi, wj = w // 4, w % 4
        for i in range(8):
            for j in range(8):
                sg[w, 8 * i + j] = g[(8 * wi + i + 4) % 32, (8 * wj + j + 4) % 32]
    return sg.reshape(8, 128)


def _pack_pairs(wT):
    """wT [cin, cout] fp32 -> [128, npair, 2, cout] fp8 (x64) pair layout."""
    cin, cout = wT.shape
    npair = cin // 256
    a = (WSC * wT).reshape(npair, 2, 128, cout).transpose(2, 0, 1, 3)
    return np.ascontiguousarray(a).astype(F8)


def _col_tiles(v, n):
    """[n*128] vector -> [128, n] tile (col c = channels 128c..128c+127)."""
    return np.ascontiguousarray(v.reshape(n, 128).T).astype(np.float32)


def _build_program(b1_pair_eq=True):
    import concourse.bass as bass
    from concourse import bacc
    import concourse.mybir as mybir
    import concourse.tile as tile

    dt = mybir.dt
    f32, bf16, f8 = dt.float32, dt.bfloat16, dt.float8e4
    AF = mybir.ActivationFunctionType
    OP = mybir.AluOpType
    DR = mybir.MatmulPerfMode.DoubleRow
    from concourse.hw_specs import get_activation_tables
    from concourse.tile_rust import add_dep_helper

    class _Bacc(bacc.Bacc):
        # Table loads are placed manually (phase-grouped); the default pass
        # picks the first table per func, ping-ponging between the Ln-only
        # and Exp-only tables every LayerNorm.
        def insert_act_table_loads(self):
            pass

    nc = _Bacc("TRN2", target_bir_lowering=False, debug=True)
    tabs = list(get_activation_tables(nc.m.arch).items())
    need_ln = {AF.Ln, AF.Exp, AF.Copy, AF.Identity}
    need_ge = {AF.Gelu, AF.Copy, AF.Identity}
    TAB_LNEXP = next(i for i, (_, s) in enumerate(tabs) if need_ln <= s)
    TAB_GELU = next(i for i, (_, s) in enumerate(tabs) if need_ge <= s)

    def _raw(i):
        return i.ins if hasattr(i, "ins") else i

    def _dep(frm, to):
        add_dep_helper(_raw(frm), _raw(to), sync=True, reason="act phase order")

    def act_table_load(set_id, after=()):
        inst = nc.scalar.add_instruction(mybir.InstLoadActFuncSet(
            name=nc.get_next_instruction_name(), act_func_set_id=set_id,
            ins=[], outs=[]))
        for a in after:
            if a is not None:
                _dep(inst, a)
        return inst
    x_d = nc.declare_dram_parameter("x", [B_LOC, TOK_IMG, DIM], bf16, isOutput=False)
    y_d = nc.declare_dram_parameter("y", [B_LOC, TOK_IMG, DIM], f32, isOutput=True)
    wq8_d = nc.declare_dram_parameter("wq8", [128, 2, 2, DIM], f8, isOutput=False)
    wk8_d = nc.declare_dram_parameter("wk8", [128, 2, 2, DIM], f8, isOutput=False)
    wv8_d = nc.declare_dram_parameter("wv8", [128, 2, 2, DIM], f8, isOutput=False)
    wp8_d = nc.declare_dram_parameter("wp8", [128, 2, 2, DIM], f8, isOutput=False)
    w18_d = nc.declare_dram_parameter("w18", [128, 2, 2, 4 * DIM], f8, isOutput=False)
    w28_d = nc.declare_dram_parameter("w28", [128, 8, 2, DIM], f8, isOutput=False)
    biass_d = nc.declare_dram_parameter("biass", [NH, 128, 128], bf16, isOutput=False)
    qm_d = nc.declare_dram_parameter("qm", [128, CHUNK], bf16, isOutput=False)
    idt_d = nc.declare_dram_parameter("idt", [128, 128], bf16, isOutput=False)
    sgw_d = nc.declare_dram_parameter("sgw", [8, 128], f32, isOutput=False)
    g1_d = nc.declare_dram_parameter("g1c", [128, 4], f32, isOutput=False)
    blp_d = nc.declare_dram_parameter("blp", [128, 4], f32, isOutput=False)
    bq64_d = nc.declare_dram_parameter("bq64", [128, 4], f32, isOutput=False)
    bks_d = nc.declare_dram_parameter("bks", [128, 4], f32, isOutput=False)
    b1g_d = nc.declare_dram_parameter("b1g", [128, 16], f32, isOutput=False)
    b2c_d = nc.declare_dram_parameter("b2c", [128, 4], f32, isOutput=False)

    from contextlib import ExitStack

    with tile.TileContext(nc) as tc:
        with ExitStack() as es:
            P = lambda *a, **kw: es.enter_context(tc.tile_pool(*a, **kw))
            wts = P(name="wts", bufs=1)
            cst = P(name="cst", bufs=1)
            xrp = P(name="xr", bufs=5)
            lnp = P(name="ln", bufs=6)
            xcbp = P(name="xcb", bufs=3)
            xnT8p = P(name="xnT8", bufs=3)
            xnTbp = P(name="xnTb", bufs=5)
            qkvp = P(name="qkv", bufs=4)
            ptp = P(name="pt", bufs=3)
            rcp = P(name="rc", bufs=3)
            pnp = P(name="pn", bufs=3)
            aoT8p = P(name="aoT8", bufs=4)
            x2Tp = P(name="x2T", bufs=3)
            x3p = P(name="x3", bufs=5)
            xc2bp = P(name="xc2b", bufs=3)
            xn2T8p = P(name="xn2T8", bufs=5)
            h18p = P(name="h18", bufs=5)
            h2Tp = P(name="h2T", bufs=3)
            yop = P(name="yo", bufs=3)
            psmm = P(name="psmm", bufs=2, space="PSUM")
            psqk = P(name="psqk", bufs=2, space="PSUM")
            psav = P(name="psav", bufs=2, space="PSUM")
            pst1 = P(name="pst1", bufs=1, space="PSUM")
            pst2 = P(name="pst2", bufs=1, space="PSUM")

            # ---- resident weights & constants ----
            WQ8 = wts.tile([128, 2, 2, DIM], f8, name="wq8")
            WK8 = wts.tile([128, 2, 2, DIM], f8, name="wk8")
            WV8 = wts.tile([128, 2, 2, DIM], f8, name="wv8")
            WP8 = wts.tile([128, 2, 2, DIM], f8, name="wp8")
            W18 = wts.tile([128, 2, 2, 4 * DIM], f8, name="w18")
            W28 = wts.tile([128, 8, 2, DIM], f8, name="w28")
            for t_, d_ in ((WQ8, wq8_d), (WK8, wk8_d), (WV8, wv8_d),
                           (WP8, wp8_d), (W18, w18_d), (W28, w28_d)):
                nc.gpsimd.dma_start(out=t_, in_=d_[:, :, :, :])

            BIASS = cst.tile([128, NH, 128], bf16, name="biass")
            nc.gpsimd.dma_start(out=BIASS, in_=biass_d[:, :, :].rearrange("h p m -> p h m"))
            QMb = cst.tile([128, CHUNK], bf16, name="qm")
            nc.gpsimd.dma_start(out=QMb, in_=qm_d[:, :])
            IDTb = cst.tile([128, 128], bf16, name="idt")
            nc.gpsimd.dma_start(out=IDTb, in_=idt_d[:, :])
            SG = cst.tile([128, 8], f32, name="sg")
            nc.gpsimd.dma_start(out=SG, in_=sgw_d[:, :].rearrange("t p -> p t"))
            G1 = cst.tile([128, 4], f32, name="g1c")
            nc.gpsimd.dma_start(out=G1, in_=g1_d[:, :])
            BLP = cst.tile([128, 4], f32, name="blp")
            nc.gpsimd.dma_start(out=BLP, in_=blp_d[:, :])
            BQ64 = cst.tile([128, 4], f32, name="bq64")
            nc.gpsimd.dma_start(out=BQ64, in_=bq64_d[:, :])
            BKS = cst.tile([128, 4], f32, name="bks")
            nc.gpsimd.dma_start(out=BKS, in_=bks_d[:, :])
            B1G = cst.tile([128, 16], f32, name="b1g")
            nc.gpsimd.dma_start(out=B1G, in_=b1g_d[:, :])
            B2C = cst.tile([128, 4], f32, name="b2c")
            nc.gpsimd.dma_start(out=B2C, in_=b2c_d[:, :])
            ONESb = cst.tile([128, 128], bf16, name="ones")
            nc.vector.memset(ONESb, 1.0)
            EPS = cst.tile([128, 1], f32, name="eps")
            nc.vector.memset(EPS, 1e-5)

            # One-time DVE touch: a tiny read of each DMA-loaded tile converts
            # DMA readiness into vector program order so downstream
            # instructions need few sync waits.
            scr = cst.tile([128, 8], f32, name="scr")
            for tt_ in (WQ8[:, 0, 0, 0:1], WK8[:, 0, 0, 0:1], WV8[:, 0, 0, 0:1],
                        WP8[:, 0, 0, 0:1], W18[:, 0, 0, 0:1], W28[:, 0, 0, 0:1]):
                nc.vector.tensor_copy(out=scr.bitcast(f8)[:, 0:1], in_=tt_)
            for tt_ in (BIASS[:, 0, 0:1], QMb[:, 0:1], IDTb[:, 0:1]):
                nc.vector.tensor_copy(out=scr.bitcast(bf16)[:, 0:1], in_=tt_)
            for tt_ in (SG, G1, BLP, BQ64, BKS, B1G, B2C):
                nc.vector.tensor_copy(out=scr[:, 0:1], in_=tt_[:, 0:1])

            ALP = nc.allow_low_precision

            # ---------------- per-chunk stages ----------------
            def load(ch):
                b, qt = ch // 4, ch % 4
                xr = [xrp.tile([128, DIM], bf16, tag=f"xr{t}", name=f"xr{t}")
                      for t in range(2)]
                for t in range(2):
                    t0 = 256 * qt + 128 * t
                    nc.sync.dma_start(out=xr[t], in_=x_d[b, t0:t0 + 128, :])
                return xr

            def ln_stats(xin, tag, after=None):
                """-> (mv [128,2] f32 mean/var, rs [128,1] f32 rstd)"""
                st = lnp.tile([128, 6], f32, tag=f"st{tag}", name=f"st{tag}")
                nc.vector.bn_stats(out=st, in_=xin)
                mv = lnp.tile([128, 2], f32, tag=f"mv{tag}", name=f"mv{tag}")
                nc.vector.bn_aggr(out=mv, in_=st)
                lg = lnp.tile([128, 1], f32, tag=f"lg{tag}", name=f"lg{tag}")
                li = nc.scalar.activation(out=lg, in_=mv[:, 1:2], func=AF.Ln,
                                          bias=EPS)
                if after is not None:
                    _dep(li, after)
                rs = lnp.tile([128, 1], f32, tag=f"rs{tag}", name=f"rs{tag}")
                nc.scalar.activation(out=rs, in_=lg, func=AF.Exp, scale=-0.5)
                return mv, rs

            def ln1_qkv(ch, xr, ld6):
                # LN1 -> xcb bf16; transpose; xnT8 fp8 (raw), xnTb' bf16
                xcb = [xcbp.tile([128, DIM], bf16, tag=f"xcb{t}", name=f"xcb{t}")
                       for t in range(2)]
                for t in range(2):
                    mv, rs = ln_stats(xr[t], "1", after=ld6)
                    with ALP(reason="normed acts bf16"):
                        nc.gpsimd.tensor_scalar(out=xcb[t], in0=xr[t],
                                                scalar1=mv[:, 0:1], scalar2=rs,
                                                op0=OP.subtract, op1=OP.mult)
                xnT8 = [xnT8p.tile([128, 2, CHUNK], f8, tag=f"xnT8{p}",
                                   name=f"xnT8{p}") for p in range(2)]
                xnTb = [xnTbp.tile([128, CHUNK], bf16, tag=f"xnTb{c}",
                                   name=f"xnTb{c}") for c in range(4)]
                xnT8_copies = []
                for c in range(4):
                    tp = pst1.tile([128, CHUNK], bf16, tag="tp1", name="tp1")
                    for t in range(2):
                        nc.tensor.transpose(tp[:, 128 * t:128 * (t + 1)],
                                            xcb[t][:, 128 * c:128 * (c + 1)], IDTb)
                    with ALP(reason="matmul input fp8"):
                        xnT8_copies.append(nc.scalar.activation(
                            out=xnT8[c // 2][:, c % 2, :], in_=tp, func=AF.Copy))
                        nc.vector.tensor_scalar(out=xnTb[c], in0=tp,
                                                scalar1=G1[:, c:c + 1],
                                                scalar2=BLP[:, c:c + 1],
                                                op0=OP.mult, op1=OP.add)
                # QKV (fp8 DoubleRow)
                qTb = [qkvp.tile([128, CHUNK], bf16, tag=f"qTb{c}", name=f"qTb{c}")
                       for c in range(4)]
                kTb = [qkvp.tile([128, CHUNK], bf16, tag=f"kTb{c}", name=f"kTb{c}")
                       for c in range(4)]
                vNb = [qkvp.tile([128, DIM], bf16, tag=f"vNb{t}", name=f"vNb{t}")
                       for t in range(2)]
                for cc in range(2):   # pairs of output c-tiles share a psum bank
                    psq = psmm.tile([128, DIM], f32, tag="mm", name="mm")
                    psk = psmm.tile([128, DIM], f32, tag="mm", name="mm")
                    for j in range(2):
                        c = 2 * cc + j
                        sl = slice(128 * c, 128 * (c + 1))
                        for p in range(2):
                            nc.tensor.matmul(psq[:, 256 * j:256 * (j + 1)],
                                             WQ8[:, p, :, sl], xnT8[p],
                                             start=(p == 0), stop=(p == 1),
                                             perf_mode=DR)
                        for p in range(2):
                            nc.tensor.matmul(psk[:, 256 * j:256 * (j + 1)],
                                             WK8[:, p, :, sl], xnT8[p],
                                             start=(p == 0), stop=(p == 1),
                                             perf_mode=DR)
                    for j in range(2):
                        c = 2 * cc + j
                        with ALP(reason="qk bf16"):
                            nc.vector.scalar_tensor_tensor(
                                out=qTb[c], in0=psq[:, 256 * j:256 * (j + 1)],
                                scalar=BQ64[:, c:c + 1], in1=QMb,
                                op0=OP.add, op1=OP.mult)
                            nc.scalar.activation(
                                out=kTb[c], in_=psk[:, 256 * j:256 * (j + 1)],
                                func=AF.Identity, bias=BKS[:, c:c + 1],
                                scale=float(SCALE / (WSC * WSC)))
                for t in range(2):
                    psv = psmm.tile([128, DIM], f32, tag="mm", name="mm")
                    for hh in range(2):
                        for p in range(2):
                            nc.tensor.matmul(
                                psv[:, 256 * hh:256 * (hh + 1)],
                                xnT8[p][:, :, 128 * t:128 * (t + 1)],
                                WV8[:, p, :, 256 * hh:256 * (hh + 1)],
                                start=(p == 0), stop=(p == 1), perf_mode=DR)
                    with ALP(reason="v bf16"):
                        nc.scalar.activation(out=vNb[t], in_=psv, func=AF.Copy,
                                             scale=float(1.0 / WSC))
                return xnTb, qTb, kTb, vNb, xnT8_copies

            def attn(ch, qTb, kTb, vNb, ld6):
                PT = [ptp.tile([128, NH * 128], bf16, tag=f"pt{wp}",
                               name=f"pt{wp}") for wp in range(2)]
                RC = [rcp.tile([128, NH * 128], bf16, tag=f"rc{wp}",
                               name=f"rc{wp}") for wp in range(2)]
                PN = [pnp.tile([128, NH * 128], bf16, tag=f"pn{wp}",
                               name=f"pn{wp}") for wp in range(2)]
                for wp in range(2):
                    sl = slice(128 * wp, 128 * (wp + 1))
                    for g in range(2):        # 4 heads per psum bank
                        qk = psqk.tile([128, 512], f32, tag="qk", name="qk")
                        for j in range(4):
                            h = 4 * g + j
                            cth, ro = h // 2, 64 * (h % 2)
                            out = qk[:, 128 * j:128 * (j + 1)]
                            nc.tensor.matmul(out, kTb[cth][ro:ro + 64, sl],
                                             qTb[cth][ro:ro + 64, sl],
                                             start=True, stop=False,
                                             skip_group_check=True)
                            nc.tensor.matmul(out, BIASS[:, h, :], IDTb,
                                             start=False, stop=True,
                                             skip_group_check=True)
                        with ALP(reason="attn weights bf16"):
                            ei = nc.scalar.activation(
                                out=PT[wp][:, 512 * g:512 * (g + 1)], in_=qk,
                                func=AF.Exp)
                            if ld6 is not None:
                                _dep(ei, ld6)
                    for g in range(2):
                        dn = psqk.tile([128, 512], f32, tag="qk", name="dn")
                        nc.tensor.matmul(dn, ONESb, PT[wp][:, 512 * g:512 * (g + 1)],
                                         start=True, stop=True)
                        with ALP(reason="attn recip bf16"):
                            nc.vector.reciprocal(
                                out=RC[wp][:, 512 * g:512 * (g + 1)], in_=dn)
                    for g in range(2):
                        with ALP(reason="attn weights bf16"):
                            nc.gpsimd.tensor_mul(
                                out=PN[wp][:, 512 * g:512 * (g + 1)],
                                in0=PT[wp][:, 512 * g:512 * (g + 1)],
                                in1=RC[wp][:, 512 * g:512 * (g + 1)])
                aoT8 = [aoT8p.tile([128, 2, CHUNK], f8, tag=f"aoT8{pp}",
                                   name=f"aoT8{pp}") for pp in range(2)]
                for hp in range(4):
                    av = psav.tile([128, CHUNK], f32, tag="av", name="av")
                    for wp in range(2):
                        for j in range(2):
                            h = 2 * hp + j
                            nc.tensor.matmul(
                                av[64 * j:64 * (j + 1), 128 * wp:128 * (wp + 1)],
                                vNb[wp][:, 64 * h:64 * (h + 1)],
                                PN[wp][:, 128 * h:128 * (h + 1)],
                                start=True, stop=True,
                                tile_position=(0, 64 * j))
                    with ALP(reason="attn out fp8"):
                        nc.scalar.activation(out=aoT8[hp // 2][:, hp % 2, :],
                                             in_=av, func=AF.Copy)
                return aoT8

            def proj_ln2(ch, xr, xnTb, aoT8):
                b, qt = ch // 4, ch % 4
                x2T = [x2Tp.tile([128, CHUNK], bf16, tag=f"x2T{c}", name=f"x2T{c}")
                       for c in range(4)]
                for cc in range(2):
                    ps = psmm.tile([128, DIM], f32, tag="mm", name="mm")
                    for j in range(2):
                        c = 2 * cc + j
                        for p in range(2):
                            nc.tensor.matmul(ps[:, 256 * j:256 * (j + 1)],
                                             WP8[:, p, :, 128 * c:128 * (c + 1)],
                                             aoT8[p], start=(p == 0), stop=(p == 1),
                                             perf_mode=DR)
                    for j in range(2):
                        c = 2 * cc + j
                        with ALP(reason="x2 bf16"):
                            nc.vector.scalar_tensor_tensor(
                                out=x2T[c], in0=ps[:, 256 * j:256 * (j + 1)],
                                scalar=float(1.0 / WSC), in1=xnTb[c],
                                op0=OP.mult, op1=OP.add)
                # back to natural + gated skip
                x3 = [x3p.tile([128, DIM], f32, tag=f"x3{t}", name=f"x3{t}")
                      for t in range(2)]
                for t in range(2):
                    tp2 = pst2.tile([128, DIM], bf16, tag="tp2", name="tp2")
                    for c in range(4):
                        nc.tensor.transpose(tp2[:, 128 * c:128 * (c + 1)],
                                            x2T[c][:, 128 * t:128 * (t + 1)], IDTb)
                    col = 2 * qt + t
                    nc.vector.scalar_tensor_tensor(
                        out=x3[t], in0=xr[t], scalar=SG[:, col:col + 1],
                        in1=tp2, op0=OP.mult, op1=OP.add)
                # LN2 -> xn2T8 (raw normalized, fp8)
                xc2b = [xc2bp.tile([128, DIM], bf16, tag=f"xc2b{t}",
                                   name=f"xc2b{t}") for t in range(2)]
                for t in range(2):
                    mv, rs = ln_stats(x3[t], "2")
                    with ALP(reason="normed acts bf16"):
                        nc.gpsimd.tensor_scalar(out=xc2b[t], in0=x3[t],
                                                scalar1=mv[:, 0:1], scalar2=rs,
                                                op0=OP.subtract, op1=OP.mult)
                xn2T8 = [xn2T8p.tile([128, 2, CHUNK], f8, tag=f"xn2T8{p}",
                                     name=f"xn2T8{p}") for p in range(2)]
                copies = []
                for c in range(4):
                    tp = pst1.tile([128, CHUNK], bf16, tag="tp1", name="tp1")
                    for t in range(2):
                        nc.tensor.transpose(tp[:, 128 * t:128 * (t + 1)],
                                            xc2b[t][:, 128 * c:128 * (c + 1)], IDTb)
                    with ALP(reason="matmul input fp8"):
                        copies.append(nc.scalar.activation(
                            out=xn2T8[c // 2][:, c % 2, :], in_=tp, func=AF.Copy))
                return x3, xn2T8, copies

            def mlp1(ch, xn2T8, ld10):
                h18 = [h18p.tile([128, 2, CHUNK], f8, tag=f"h18{m}",
                                 name=f"h18{m}") for m in range(8)]
                last_gelu = None
                for m in range(8):
                    ps = psmm.tile([128, DIM], f32, tag="mm", name="mm")
                    for j in range(2):
                        o = 2 * m + j
                        for p in range(2):
                            nc.tensor.matmul(ps[:, 256 * j:256 * (j + 1)],
                                             W18[:, p, :, 128 * o:128 * (o + 1)],
                                             xn2T8[p], start=(p == 0), stop=(p == 1),
                                             perf_mode=DR)
                    with ALP(reason="mlp hidden fp8"):
                        if b1_pair_eq:
                            gi = nc.scalar.activation(
                                out=h18[m][:, :, :],
                                in_=ps[:, :].rearrange("p (j t) -> p j t", j=2),
                                func=AF.Gelu, bias=B1G[:, 2 * m:2 * m + 1],
                                scale=float(1.0 / WSC))
                            _dep(gi, ld10)
                            last_gelu = gi
                        else:
                            for j in range(2):
                                gi = nc.scalar.activation(
                                    out=h18[m][:, j, :],
                                    in_=ps[:, 256 * j:256 * (j + 1)], func=AF.Gelu,
                                    bias=B1G[:, 2 * m + j:2 * m + j + 1],
                                    scale=float(1.0 / WSC))
                                _dep(gi, ld10)
                                last_gelu = gi
                return h18, last_gelu

            def mlp2_store(ch, x3, h18):
                b, qt = ch // 4, ch % 4
                h2T = [h2Tp.tile([128, CHUNK], bf16, tag=f"h2T{c}", name=f"h2T{c}")
                       for c in range(4)]
                for cc in range(2):
                    ps = psmm.tile([128, DIM], f32, tag="mm", name="mm")
                    for j in range(2):
                        c = 2 * cc + j
                        for mp in range(8):
                            nc.tensor.matmul(ps[:, 256 * j:256 * (j + 1)],
                                             W28[:, mp, :, 128 * c:128 * (c + 1)],
                                             h18[mp], start=(mp == 0), stop=(mp == 7),
                                             perf_mode=DR)
                    for j in range(2):
                        c = 2 * cc + j
                        with ALP(reason="mlp out bf16"):
                            nc.vector.tensor_scalar(
                                out=h2T[c], in0=ps[:, 256 * j:256 * (j + 1)],
                                scalar1=float(1.0 / WSC),
                                scalar2=B2C[:, c:c + 1], op0=OP.mult, op1=OP.add)
                for t in range(2):
                    tp4 = pst2.tile([128, DIM], bf16, tag="tp2", name="tp2")
                    for c in range(4):
                        nc.tensor.transpose(tp4[:, 128 * c:128 * (c + 1)],
                                            h2T[c][:, 128 * t:128 * (t + 1)], IDTb)
                    yo = yop.tile([128, DIM], f32, tag=f"yo{t}", name=f"yo{t}")
                    nc.vector.tensor_add(out=yo, in0=x3[t], in1=tp4)
                    t0 = 256 * qt + 128 * t
                    nc.sync.dma_start(out=y_d[b, t0:t0 + 128, :], in_=yo)

            # -------- software-pipelined chunk-pair loop --------
            # Pair p+1's loads+LN1+QKV are emitted before pair p's MLP phase
            # so DVE/PE have queued work while ACT runs the gelu table.
            G = 4
            NGRP = NCHUNK // G
            st = {}

            def phase1(gr, ld6):
                # loads + LN1 + QKV for all chunks of group gr
                copies = []
                for ch in range(G * gr, G * gr + G):
                    xr = load(ch)
                    xnTb, qTb, kTb, vNb, cps = ln1_qkv(ch, xr, ld6)
                    st[ch] = [xr, xnTb, qTb, kTb, vNb]
                    copies += cps
                return copies

            ld6 = act_table_load(TAB_LNEXP)
            phase1(0, ld6)
            for pr in range(NGRP):
                chs = tuple(range(G * pr, G * pr + G))
                for ch in chs:
                    xr, xnTb, qTb, kTb, vNb = st[ch]
                    st[ch].append(attn(ch, qTb, kTb, vNb, ld6))
                copies = []
                for ch in chs:
                    xr, xnTb, qTb, kTb, vNb, aoT8 = st[ch]
                    x3, xn2T8, cps = proj_ln2(ch, xr, xnTb, aoT8)
                    st[ch] += [x3, xn2T8]
                    copies += cps
                if pr + 1 < NGRP:
                    copies += phase1(pr + 1, ld6)
                ld10 = act_table_load(TAB_GELU, after=copies)
                for ch in chs:
                    h18, last_gelu = mlp1(ch, st[ch][7], ld10)
                    st[ch].append(h18)
                for ch in chs:
                    x3, h18 = st[ch][6], st[ch][8]
                    mlp2_store(ch, x3, h18)
                    del st[ch]
                ld6 = act_table_load(TAB_LNEXP, after=(last_gelu,))

    nc.compile()
    return nc


def _host_consts(rel_table):
    """BIASS [8,128,128] bf16 (n,m orientation, block-diag NEG, masked-q rows
    zeroed) and QMb keep-mask [128, CHUNK] bf16."""
    idx = _rel_index(WS).reshape(-1)
    bias = rel_table.reshape(-1, NH)[idx].reshape(N, NH, N)  # [n, h, m]
    qmask = _shift_mask(WS, SHIFT)
    keep = (~qmask).astype(np.float32)
    biass = np.full((NH, 128, 128), NEG, np.float32)
    for h in range(NH):
        bnm = bias[:, h, :] * keep[:, None]       # [n, m], masked-q rows -> 0
        biass[h, :64, :64] = bnm
        biass[h, 64:, 64:] = bnm
    qm = np.tile(keep, CHUNK // N)[None, :].repeat(128, 0)
    return biass.astype(ml_dtypes.bfloat16), qm.astype(ml_dtypes.bfloat16)


def kernel(**inputs):
    from concourse.bass_utils import run_bass_kernel_spmd

    f32 = np.float32
    x = np.asarray(inputs["x"], f32)
    g1 = np.asarray(inputs["ln1_g"], f32)
    bl1 = np.asarray(inputs["ln1_b"], f32)
    g2 = np.asarray(inputs["ln2_g"], f32)
    bl2 = np.asarray(inputs["ln2_b"], f32)
    wq = np.asarray(inputs["wq"], f32)
    wk = np.asarray(inputs["wk"], f32)
    wv = np.asarray(inputs["wv"], f32)
    wp = np.asarray(inputs["wp"], f32)
    bq = np.asarray(inputs["bq"], f32)
    bk = np.asarray(inputs["bk"], f32)
    bv = np.asarray(inputs["bv"], f32)
    bp = np.asarray(inputs["bp"], f32)
    w1 = np.asarray(inputs["mlp_w1"], f32)
    b1 = np.asarray(inputs["mlp_b1"], f32)
    w2 = np.asarray(inputs["mlp_w2"], f32)
    b2 = np.asarray(inputs["mlp_b2"], f32)

    # fold LN1 gamma into QKV weights; beta into their biases
    wq_e = wq * g1[None, :]
    wk_e = wk * g1[None, :]
    wv_e = wv * g1[None, :]
    bq_e = bq + wq_e @ bl1
    bk_e = bk + wk_e @ bl1
    bv_e = bv + wv_e @ bl1
    # proj bias absorbs bv (attn rows sum to 1) -> blp rides the residual
    bp3 = bp + wp @ bv_e
    blp = bl1 + bp3
    # fold LN2 gamma/beta into MLP1
    w1_e = w1 * g2[None, :]
    b1_e = b1 + w1_e @ bl2

    biass, qm = _host_consts(np.asarray(inputs["rel_table"], f32))
    sgw = _win_order_sigmoid_gate(np.asarray(inputs["gate"], f32))
    b1g = _col_tiles(b1_e, 16)
    b1_pair_eq = bool(np.allclose(b1g[:, 0::2], b1g[:, 1::2]))

    common = {
        "wq8": _pack_pairs(np.ascontiguousarray(wq_e.T)),
        "wk8": _pack_pairs(np.ascontiguousarray(wk_e.T)),
        "wv8": _pack_pairs(np.ascontiguousarray(wv_e.T)),
        "wp8": _pack_pairs(np.ascontiguousarray(wp.T)),
        "w18": _pack_pairs(np.ascontiguousarray(w1_e.T)),
        "w28": _pack_pairs(np.ascontiguousarray(w2.T)).reshape(128, 8, 2, DIM),
        "biass": biass, "qm": qm,
        "idt": np.eye(128, dtype=ml_dtypes.bfloat16),
        "sgw": sgw,
        "g1c": _col_tiles(g1, 4),
        "blp": _col_tiles(blp, 4),
        "bq64": _col_tiles(WSC * bq_e, 4),
        "bks": _col_tiles(SCALE * bk_e / WSC, 4),
        "b1g": b1g,
        "b2c": _col_tiles(b2, 4),
    }

    if "prog" not in _prog_cache:
        _prog_cache["prog"] = _build_program(b1_pair_eq)
    nc = _prog_cache["prog"]

    perm = _perm_idx()
    xw = x.reshape(B_TOTAL, TOK_IMG, DIM)[:, perm, :]
    xw8 = xw.astype(ml_dtypes.bfloat16)
    in_maps = []
    for c in range(NCORES):
        m = dict(common)
        m["x"] = np.ascontiguousarray(xw8[c * B_LOC:(c + 1) * B_LOC])
        in_maps.append(m)
    res = run_bass_kernel_spmd(nc, in_maps, core_ids=list(range(NCORES)))
    yw = np.concatenate([res.results[c]["y"] for c in range(NCORES)], axis=0)
    out = np.empty((B_TOTAL, TOK_IMG, DIM), np.float32)
    out[:, perm, :] = yw
    return out.reshape(B_TOTAL, 1, HRES, WRES, DIM).astype(np.float32)


# revision 24
# speedup vs baseline: 2.6101x; 1.0070x over previous
"""CloudCastV2 shifted-window transformer block on 8 trn2 NeuronCores.

Data-parallel over batch: 64 images -> 8 per core. The (-4,-4) roll + 8x8
window partition is folded into host-side permutation; on chip everything is
window-ordered token space (8192 tokens x 512 ch per core), processed in 32
chunks of 256 tokens (4 windows / 2 window-pairs).

v2 design:
  - all dense matmuls (QKV / proj / MLP) run in fp8(e4m3) DoubleRow perf
    mode: weights are scaled x64 on host, the 1/64 is folded into the
    PSUM->SBUF copy scale. LN gammas/betas folded into weights/biases.
  - transposes are bf16 PE transposes (1 cycle/row); activations live in
    bf16, residual stream in f32.
  - attention: qk^T (bf16) + relative-position bias added via a second
    matmul (bias stationary, identity moving) -> ACT Exp -> ones-matmul
    denominator -> DVE reciprocal -> pn = PT*rc (bf16 2x) -> AV.
  - rstd = exp(-0.5*ln(var+eps)) so LN shares the Exp ACT table; only the
    MLP Gelu needs a second table. Chunks are processed in groups of 4
    with phase-grouped ACT usage (manually placed LoadActFuncSet + same-
    engine deps) -> 2 table swaps per 4 chunks; group g+1's loads+LN1+QKV
    are emitted before group g's MLP phase (software pipelining).
  - elementwise work balanced across DVE / ACT / Pool; x/y DMAs issued from
    the SP sequencer via HWDGE (Pool only issues the one-time weight loads).
"""

import numpy as np
import ml_dtypes

WS, SHIFT, HEADS, DIM, HRES, WRES = 8, 4, 8, 512, 32, 32
N = WS * WS
NH = HEADS
D = DIM // NH
B_TOTAL, NCORES = 64, 8
B_LOC = B_TOTAL // NCORES
TOK_IMG = HRES * WRES
CHUNK = 256
NCHUNK = B_LOC * TOK_IMG // CHUNK   # 32
SCALE = float(D) ** -0.5
NEG = -30000.0
WSC = 64.0                           # fp8 weight prescale
F8 = ml_dtypes.float8_e4m3

_prog_cache = {}


def _rel_index(ws):
    coords = np.arange(ws)
    grid = np.stack(np.meshgrid(coords, coords, indexing="ij"))
    flat = grid.reshape(2, -1)
    rel = flat[:, :, None] - flat[:, None, :]
    rel[0] += ws - 1
    rel[1] += ws - 1
    return rel[0] * (2 * ws - 1) + rel[1]


def _shift_mask(ws, shift):
    base = np.zeros((ws, ws), dtype=bool)
    base[ws - shift:, :] = True
    base[:, ws - shift:] = True
    return base.reshape(-1)


def _win_pieces(w):
    wi, wj = w // 4, w % 4
    ih = [(0, 8, 8 * wi + 4)] if wi < 3 else [(0, 4, 28), (4, 4, 0)]
    jw = [(0, 8, 8 * wj + 4)] if wj < 3 else [(0, 4, 28), (4, 4, 0)]
    out = []
    for (i0, ni, h0) in ih:
        for (j0, nj, w0) in jw:
            out.append((i0, ni, h0, j0, nj, w0))
    return out


_PERM = None


def _perm_idx():
    global _PERM
    if _PERM is None:
        p = np.zeros(1024, np.int64)
        for w in range(16):
            for (i0, ni, h0, j0, nj, w0) in _win_pieces(w):
                for a in range(ni):
                    for bb in range(nj):
                        p[64 * w + 8 * (i0 + a) + (j0 + bb)] = (h0 + a) * WRES + (w0 + bb)
        _PERM = p
    return _PERM


def _win_order_sigmoid_gate(gate):
    g = 1.0 / (1.0 + np.exp(-gate.reshape(HRES, WRES).astype(np.float64)))
    g = g.astype(np.float32)
    sg = np.zeros((16, 64), np.float32)
    for w in range(16):
        wi, wj = w // 4, w % 4
        for i in range(8):
            for j in range(8):
                sg[w, 8 * i + j] = g[(8 * wi + i + 4) % 32, (8 * wj + j + 4) % 32]
    return sg.reshape(8, 128)


def _pack_pairs(wT):
    """wT [cin, cout] fp32 -> [128, npair, 2, cout] fp8 (x64) pair layout."""
    cin, cout = wT.shape
    npair = cin // 256
    a = (WSC * wT).reshape(npair, 2, 128, cout).transpose(2, 0, 1, 3)
    return np.ascontiguousarray(a).astype(F8)


def _col_tiles(v, n):
    """[n*128] vector -> [128, n] tile (col c = channels 128c..128c+127)."""
    return np.ascontiguousarray(v.reshape(n, 128).T).astype(np.float32)


def _build_program(b1_pair_eq=True):
    import concourse.bass as bass
    from concourse import bacc
    import concourse.mybir as mybir
    import concourse.tile as tile

    dt = mybir.dt
    f32, bf16, f8 = dt.float32, dt.bfloat16, dt.float8e4
    AF = mybir.ActivationFunctionType
    OP = mybir.AluOpType
    DR = mybir.MatmulPerfMode.DoubleRow
    from concourse.hw_specs import get_activation_tables
    from concourse.tile_rust import add_dep_helper

    class _Bacc(bacc.Bacc):
        # Table loads are placed manually (phase-grouped); the default pass
        # picks the first table per func, ping-ponging between the Ln-only
        # and Exp-only tables every LayerNorm.
        def insert_act_table_loads(self):
            pass

    nc = _Bacc("TRN2", target_bir_lowering=False, debug=True)
    tabs = list(get_activation_tables(nc.m.arch).items())
    need_ln = {AF.Ln, AF.Exp, AF.Copy, AF.Identity}
    need_ge = {AF.Gelu, AF.Copy, AF.Identity}
    TAB_LNEXP = next(i for i, (_, s) in enumerate(tabs) if need_ln <= s)
    TAB_GELU = next(i for i, (_, s) in enumerate(tabs) if need_ge <= s)

    def _raw(i):
        return i.ins if hasattr(i, "ins") else i

    def _dep(frm, to):
        add_dep_helper(_raw(frm), _raw(to), sync=True, reason="act phase order")

    def act_table_load(set_id, after=()):
        inst = nc.scalar.add_instruction(mybir.InstLoadActFuncSet(
            name=nc.get_next_instruction_name(), act_func_set_id=set_id,
            ins=[], outs=[]))
        for a in after:
            if a is not None:
                _dep(inst, a)
        return inst
    x_d = nc.declare_dram_parameter("x", [B_LOC, TOK_IMG, DIM], bf16, isOutput=False)
    y_d = nc.declare_dram_parameter("y", [B_LOC, TOK_IMG, DIM], f32, isOutput=True)
    wq8_d = nc.declare_dram_parameter("wq8", [128, 2, 2, DIM], f8, isOutput=False)
    wk8_d = nc.declare_dram_parameter("wk8", [128, 2, 2, DIM], f8, isOutput=False)
    wv8_d = nc.declare_dram_parameter("wv8", [128, 2, 2, DIM], f8, isOutput=False)
    wp8_d = nc.declare_dram_parameter("wp8", [128, 2, 2, DIM], f8, isOutput=False)
    w18_d = nc.declare_dram_parameter("w18", [128, 2, 2, 4 * DIM], f8, isOutput=False)
    w28_d = nc.declare_dram_parameter("w28", [128, 8, 2, DIM], f8, isOutput=False)
    biass_d = nc.declare_dram_parameter("biass", [NH, 128, 128], bf16, isOutput=False)
    qm_d = nc.declare_dram_parameter("qm", [128, CHUNK], bf16, isOutput=False)
    idt_d = nc.declare_dram_parameter("idt", [128, 128], bf16, isOutput=False)
    sgw_d = nc.declare_dram_parameter("sgw", [8, 128], f32, isOutput=False)
    g1_d = nc.declare_dram_parameter("g1c", [128, 4], f32, isOutput=False)
    blp_d = nc.declare_dram_parameter("blp", [128, 4], f32, isOutput=False)
    bq64_d = nc.declare_dram_parameter("bq64", [128, 4], f32, isOutput=False)
    bks_d = nc.declare_dram_parameter("bks", [128, 4], f32, isOutput=False)
    b1g_d = nc.declare_dram_parameter("b1g", [128, 16], f32, isOutput=False)
    b2c_d = nc.declare_dram_parameter("b2c", [128, 4], f32, isOutput=False)

    from contextlib import ExitStack

    with tile.TileContext(nc) as tc:
        with ExitStack() as es:
            P = lambda *a, **kw: es.enter_context(tc.tile_pool(*a, **kw))
            wts = P(name="wts", bufs=1)
            cst = P(name="cst", bufs=1)
            xrp = P(name="xr", bufs=5)
            lnp = P(name="ln", bufs=6)
            xcbp = P(name="xcb", bufs=3)
            xnT8p = P(name="xnT8", bufs=3)
            xnTbp = P(name="xnTb", bufs=5)
            qkvp = P(name="qkv", bufs=4)
            ptp = P(name="pt", bufs=3)
            rcp = P(name="rc", bufs=3)
            pnp = P(name="pn", bufs=3)
            aoT8p = P(name="aoT8", bufs=4)
            x2Tp = P(name="x2T", bufs=3)
            x3p = P(name="x3", bufs=5)
            xc2bp = P(name="xc2b", bufs=3)
            xn2T8p = P(name="xn2T8", bufs=5)
            h18p = P(name="h18", bufs=5)
            h2Tp = P(name="h2T", bufs=3)
            yop = P(name="yo", bufs=3)
            psmm = P(name="psmm", bufs=2, space="PSUM")
            psqk = P(name="psqk", bufs=2, space="PSUM")
            psav = P(name="psav", bufs=2, space="PSUM")
            pst1 = P(name="pst1", bufs=2, space="PSUM")
            pst2 = pst1

            # ---- resident weights & constants ----
            WQ8 = wts.tile([128, 2, 2, DIM], f8, name="wq8")
            WK8 = wts.tile([128, 2, 2, DIM], f8, name="wk8")
            WV8 = wts.tile([128, 2, 2, DIM], f8, name="wv8")
            WP8 = wts.tile([128, 2, 2, DIM], f8, name="wp8")
            W18 = wts.tile([128, 2, 2, 4 * DIM], f8, name="w18")
            W28 = wts.tile([128, 8, 2, DIM], f8, name="w28")
            for t_, d_ in ((WQ8, wq8_d), (WK8, wk8_d), (WV8, wv8_d),
                           (WP8, wp8_d), (W18, w18_d), (W28, w28_d)):
                nc.gpsimd.dma_start(out=t_, in_=d_[:, :, :, :])

            BIASS = cst.tile([128, NH, 128], bf16, name="biass")
            nc.gpsimd.dma_start(out=BIASS, in_=biass_d[:, :, :].rearrange("h p m -> p h m"))
            QMb = cst.tile([128, CHUNK], bf16, name="qm")
            nc.gpsimd.dma_start(out=QMb, in_=qm_d[:, :])
            IDTb = cst.tile([128, 128], bf16, name="idt")
            nc.gpsimd.dma_start(out=IDTb, in_=idt_d[:, :])
            SG = cst.tile([128, 8], f32, name="sg")
            nc.gpsimd.dma_start(out=SG, in_=sgw_d[:, :].rearrange("t p -> p t"))
            G1 = cst.tile([128, 4], f32, name="g1c")
            nc.gpsimd.dma_start(out=G1, in_=g1_d[:, :])
            BLP = cst.tile([128, 4], f32, name="blp")
            nc.gpsimd.dma_start(out=BLP, in_=blp_d[:, :])
            BQ64 = cst.tile([128, 4], f32, name="bq64")
            nc.gpsimd.dma_start(out=BQ64, in_=bq64_d[:, :])
            BKS = cst.tile([128, 4], f32, name="bks")
            nc.gpsimd.dma_start(out=BKS, in_=bks_d[:, :])
            B1G = cst.tile([128, 16], f32, name="b1g")
            nc.gpsimd.dma_start(out=B1G, in_=b1g_d[:, :])
            B2C = cst.tile([128, 4], f32, name="b2c")
            nc.gpsimd.dma_start(out=B2C, in_=b2c_d[:, :])
            ONESb = cst.tile([128, 128], bf16, name="ones")
            nc.vector.memset(ONESb, 1.0)
            EPS = cst.tile([128, 1], f32, name="eps")
            nc.vector.memset(EPS, 1e-5)

            # One-time DVE touch: a tiny read of each DMA-loaded tile converts
            # DMA readiness into vector program order so downstream
            # instructions need few sync waits.
            scr = cst.tile([128, 8], f32, name="scr")
            for tt_ in (WQ8[:, 0, 0, 0:1], WK8[:, 0, 0, 0:1], WV8[:, 0, 0, 0:1],
                        WP8[:, 0, 0, 0:1], W18[:, 0, 0, 0:1], W28[:, 0, 0, 0:1]):
                nc.vector.tensor_copy(out=scr.bitcast(f8)[:, 0:1], in_=tt_)
            for tt_ in (BIASS[:, 0, 0:1], QMb[:, 0:1], IDTb[:, 0:1]):
                nc.vector.tensor_copy(out=scr.bitcast(bf16)[:, 0:1], in_=tt_)
            for tt_ in (SG, G1, BLP, BQ64, BKS, B1G, B2C):
                nc.vector.tensor_copy(out=scr[:, 0:1], in_=tt_[:, 0:1])

            ALP = nc.allow_low_precision

            # ---------------- per-chunk stages ----------------
            def load(ch):
                b, qt = ch // 4, ch % 4
                xr = [xrp.tile([128, DIM], bf16, tag=f"xr{t}", name=f"xr{t}")
                      for t in range(2)]
                for t in range(2):
                    t0 = 256 * qt + 128 * t
                    nc.sync.dma_start(out=xr[t], in_=x_d[b, t0:t0 + 128, :])
                return xr

            def ln_stats(xin, tag, after=None):
                """-> (mv [128,2] f32 mean/var, rs [128,1] f32 rstd)"""
                st = lnp.tile([128, 6], f32, tag=f"st{tag}", name=f"st{tag}")
                nc.vector.bn_stats(out=st, in_=xin)
                mv = lnp.tile([128, 2], f32, tag=f"mv{tag}", name=f"mv{tag}")
                nc.vector.bn_aggr(out=mv, in_=st)
                lg = lnp.tile([128, 1], f32, tag=f"lg{tag}", name=f"lg{tag}")
                li = nc.scalar.activation(out=lg, in_=mv[:, 1:2], func=AF.Ln,
                                          bias=EPS)
                if after is not None:
                    _dep(li, after)
                rs = lnp.tile([128, 1], f32, tag=f"rs{tag}", name=f"rs{tag}")
                nc.scalar.activation(out=rs, in_=lg, func=AF.Exp, scale=-0.5)
                return mv, rs

            def ln1_qkv(ch, xr, ld6):
                # LN1 -> xcb bf16; transpose; xnT8 fp8 (raw), xnTb' bf16
                xcb = [xcbp.tile([128, DIM], bf16, tag=f"xcb{t}", name=f"xcb{t}")
                       for t in range(2)]
                for t in range(2):
                    mv, rs = ln_stats(xr[t], "1", after=ld6)
                    with ALP(reason="normed acts bf16"):
                        nc.gpsimd.tensor_scalar(out=xcb[t], in0=xr[t],
                                                scalar1=mv[:, 0:1], scalar2=rs,
                                                op0=OP.subtract, op1=OP.mult)
                xnT8 = [xnT8p.tile([128, 2, CHUNK], f8, tag=f"xnT8{p}",
                                   name=f"xnT8{p}") for p in range(2)]
                xnTb = [xnTbp.tile([128, CHUNK], bf16, tag=f"xnTb{c}",
                                   name=f"xnTb{c}") for c in range(4)]
                xnT8_copies = []
                for c in range(4):
                    tp = pst1.tile([128, CHUNK], bf16, tag="tp", name="tp")
                    for t in range(2):
                        nc.tensor.transpose(tp[:, 128 * t:128 * (t + 1)],
                                            xcb[t][:, 128 * c:128 * (c + 1)], IDTb)
                    with ALP(reason="matmul input fp8"):
                        xnT8_copies.append(nc.scalar.activation(
                            out=xnT8[c // 2][:, c % 2, :], in_=tp, func=AF.Copy))
                        nc.vector.tensor_scalar(out=xnTb[c], in0=tp,
                                                scalar1=G1[:, c:c + 1],
                                                scalar2=BLP[:, c:c + 1],
                                                op0=OP.mult, op1=OP.add)
                # QKV (fp8 DoubleRow)
                qTb = [qkvp.tile([128, CHUNK], bf16, tag=f"qTb{c}", name=f"qTb{c}")
                       for c in range(4)]
                kTb = [qkvp.tile([128, CHUNK], bf16, tag=f"kTb{c}", name=f"kTb{c}")
                       for c in range(4)]
                vNb = [qkvp.tile([128, DIM], bf16, tag=f"vNb{t}", name=f"vNb{t}")
                       for t in range(2)]
                for cc in range(2):   # pairs of output c-tiles share a psum bank
                    psq = psmm.tile([128, DIM], f32, tag="mm", name="mm")
                    psk = psmm.tile([128, DIM], f32, tag="mm", name="mm")
                    for j in range(2):
                        c = 2 * cc + j
                        sl = slice(128 * c, 128 * (c + 1))
                        for p in range(2):
                            nc.tensor.matmul(psq[:, 256 * j:256 * (j + 1)],
                                             WQ8[:, p, :, sl], xnT8[p],
                                             start=(p == 0), stop=(p == 1),
                                             perf_mode=DR)
                        for p in range(2):
                            nc.tensor.matmul(psk[:, 256 * j:256 * (j + 1)],
                                             WK8[:, p, :, sl], xnT8[p],
                                             start=(p == 0), stop=(p == 1),
                                             perf_mode=DR)
                    for j in range(2):
                        c = 2 * cc + j
                        with ALP(reason="qk bf16"):
                            nc.vector.scalar_tensor_tensor(
                                out=qTb[c], in0=psq[:, 256 * j:256 * (j + 1)],
                                scalar=BQ64[:, c:c + 1], in1=QMb,
                                op0=OP.add, op1=OP.mult)
                            nc.scalar.activation(
                                out=kTb[c], in_=psk[:, 256 * j:256 * (j + 1)],
                                func=AF.Identity, bias=BKS[:, c:c + 1],
                                scale=float(SCALE / (WSC * WSC)))
                for t in range(2):
                    psv = psmm.tile([128, DIM], f32, tag="mm", name="mm")
                    for hh in range(2):
                        for p in range(2):
                            nc.tensor.matmul(
                                psv[:, 256 * hh:256 * (hh + 1)],
                                xnT8[p][:, :, 128 * t:128 * (t + 1)],
                                WV8[:, p, :, 256 * hh:256 * (hh + 1)],
                                start=(p == 0), stop=(p == 1), perf_mode=DR)
                    with ALP(reason="v bf16"):
                        nc.scalar.activation(out=vNb[t], in_=psv, func=AF.Copy,
                                             scale=float(1.0 / WSC))
                return xnTb, qTb, kTb, vNb, xnT8_copies

            def attn(ch, qTb, kTb, vNb, ld6):
                PT = [ptp.tile([128, NH * 128], bf16, tag=f"pt{wp}",
                               name=f"pt{wp}") for wp in range(2)]
                RC = [rcp.tile([128, NH * 128], bf16, tag=f"rc{wp}",
                               name=f"rc{wp}") for wp in range(2)]
                PN = [pnp.tile([128, NH * 128], bf16, tag=f"pn{wp}",
                               name=f"pn{wp}") for wp in range(2)]
                for wp in range(2):
                    sl = slice(128 * wp, 128 * (wp + 1))
                    for g in range(2):        # 4 heads per psum bank
                        qk = psqk.tile([128, 512], f32, tag="qk", name="qk")
                        for j in range(4):
                            h = 4 * g + j
                            cth, ro = h // 2, 64 * (h % 2)
                            out = qk[:, 128 * j:128 * (j + 1)]
                            nc.tensor.matmul(out, kTb[cth][ro:ro + 64, sl],
                                             qTb[cth][ro:ro + 64, sl],
                                             start=True, stop=False,
                                             skip_group_check=True)
                            nc.tensor.matmul(out, BIASS[:, h, :], IDTb,
                                             start=False, stop=True,
                                             skip_group_check=True)
                        with ALP(reason="attn weights bf16"):
                            ei = nc.scalar.activation(
                                out=PT[wp][:, 512 * g:512 * (g + 1)], in_=qk,
                                func=AF.Exp)
                            if ld6 is not None:
                                _dep(ei, ld6)
                    for g in range(2):
                        dn = psqk.tile([128, 512], f32, tag="qk", name="dn")
                        nc.tensor.matmul(dn, ONESb, PT[wp][:, 512 * g:512 * (g + 1)],
                                         start=True, stop=True)
                        with ALP(reason="attn recip bf16"):
                            nc.vector.reciprocal(
                                out=RC[wp][:, 512 * g:512 * (g + 1)], in_=dn)
                    for g in range(2):
                        with ALP(reason="attn weights bf16"):
                            nc.gpsimd.tensor_mul(
                                out=PN[wp][:, 512 * g:512 * (g + 1)],
                                in0=PT[wp][:, 512 * g:512 * (g + 1)],
                                in1=RC[wp][:, 512 * g:512 * (g + 1)])
                aoT8 = [aoT8p.tile([128, 2, CHUNK], f8, tag=f"aoT8{pp}",
                                   name=f"aoT8{pp}") for pp in range(2)]
                for hp in range(4):
                    av = psav.tile([128, CHUNK], f32, tag="av", name="av")
                    for wp in range(2):
                        for j in range(2):
                            h = 2 * hp + j
                            nc.tensor.matmul(
                                av[64 * j:64 * (j + 1), 128 * wp:128 * (wp + 1)],
                                vNb[wp][:, 64 * h:64 * (h + 1)],
                                PN[wp][:, 128 * h:128 * (h + 1)],
                                start=True, stop=True,
                                tile_position=(0, 64 * j))
                    with ALP(reason="attn out fp8"):
                        nc.scalar.activation(out=aoT8[hp // 2][:, hp % 2, :],
                                             in_=av, func=AF.Copy)
                return aoT8

            def proj_ln2(ch, xr, xnTb, aoT8):
                b, qt = ch // 4, ch % 4
                x2T = [x2Tp.tile([128, CHUNK], bf16, tag=f"x2T{c}", name=f"x2T{c}")
                       for c in range(4)]
                for cc in range(2):
                    ps = psmm.tile([128, DIM], f32, tag="mm", name="mm")
                    for j in range(2):
                        c = 2 * cc + j
                        for p in range(2):
                            nc.tensor.matmul(ps[:, 256 * j:256 * (j + 1)],
                                             WP8[:, p, :, 128 * c:128 * (c + 1)],
                                             aoT8[p], start=(p == 0), stop=(p == 1),
                                             perf_mode=DR)
                    for j in range(2):
                        c = 2 * cc + j
                        with ALP(reason="x2 bf16"):
                            nc.vector.scalar_tensor_tensor(
                                out=x2T[c], in0=ps[:, 256 * j:256 * (j + 1)],
                                scalar=float(1.0 / WSC), in1=xnTb[c],
                                op0=OP.mult, op1=OP.add)
                # back to natural + gated skip
                x3 = [x3p.tile([128, DIM], f32, tag=f"x3{t}", name=f"x3{t}")
                      for t in range(2)]
                for t in range(2):
                    tp2 = pst2.tile([128, DIM], bf16, tag="tp", name="tp")
                    for c in range(4):
                        nc.tensor.transpose(tp2[:, 128 * c:128 * (c + 1)],
                                            x2T[c][:, 128 * t:128 * (t + 1)], IDTb)
                    col = 2 * qt + t
                    nc.vector.scalar_tensor_tensor(
                        out=x3[t], in0=xr[t], scalar=SG[:, col:col + 1],
                        in1=tp2, op0=OP.mult, op1=OP.add)
                # LN2 -> xn2T8 (raw normalized, fp8)
                xc2b = [xc2bp.tile([128, DIM], bf16, tag=f"xc2b{t}",
                                   name=f"xc2b{t}") for t in range(2)]
                for t in range(2):
                    mv, rs = ln_stats(x3[t], "2")
                    with ALP(reason="normed acts bf16"):
                        nc.gpsimd.tensor_scalar(out=xc2b[t], in0=x3[t],
                                                scalar1=mv[:, 0:1], scalar2=rs,
                                                op0=OP.subtract, op1=OP.mult)
                xn2T8 = [xn2T8p.tile([128, 2, CHUNK], f8, tag=f"xn2T8{p}",
                                     name=f"xn2T8{p}") for p in range(2)]
                copies = []
                for c in range(4):
                    tp = pst1.tile([128, CHUNK], bf16, tag="tp", name="tp")
                    for t in range(2):
                        nc.tensor.transpose(tp[:, 128 * t:128 * (t + 1)],
                                            xc2b[t][:, 128 * c:128 * (c + 1)], IDTb)
                    with ALP(reason="matmul input fp8"):
                        copies.append(nc.vector.tensor_copy(
                            out=xn2T8[c // 2][:, c % 2, :], in_=tp))
                return x3, xn2T8, copies

            def mlp1(ch, xn2T8, ld10):
                h18 = [h18p.tile([128, 2, CHUNK], f8, tag=f"h18{m}",
                                 name=f"h18{m}") for m in range(8)]
                last_gelu = None
                for m in range(8):
                    ps = psmm.tile([128, DIM], f32, tag="mm", name="mm")
                    for j in range(2):
                        o = 2 * m + j
                        for p in range(2):
                            nc.tensor.matmul(ps[:, 256 * j:256 * (j + 1)],
                                             W18[:, p, :, 128 * o:128 * (o + 1)],
                                             xn2T8[p], start=(p == 0), stop=(p == 1),
                                             perf_mode=DR)
                    with ALP(reason="mlp hidden fp8"):
                        if b1_pair_eq:
                            gi = nc.scalar.activation(
                                out=h18[m][:, :, :],
                                in_=ps[:, :].rearrange("p (j t) -> p j t", j=2),
                                func=AF.Gelu, bias=B1G[:, 2 * m:2 * m + 1],
                                scale=float(1.0 / WSC))
                            _dep(gi, ld10)
                            last_gelu = gi
                        else:
                            for j in range(2):
                                gi = nc.scalar.activation(
                                    out=h18[m][:, j, :],
                                    in_=ps[:, 256 * j:256 * (j + 1)], func=AF.Gelu,
                                    bias=B1G[:, 2 * m + j:2 * m + j + 1],
                                    scale=float(1.0 / WSC))
                                _dep(gi, ld10)
                                last_gelu = gi
                return h18, last_gelu

            def mlp2_store(ch, x3, h18):
                b, qt = ch // 4, ch % 4
                h2T = [h2Tp.tile([128, CHUNK], bf16, tag=f"h2T{c}", name=f"h2T{c}")
                       for c in range(4)]
                for cc in range(2):
                    ps = psmm.tile([128, DIM], f32, tag="mm", name="mm")
                    for j in range(2):
                        c = 2 * cc + j
                        for mp in range(8):
                            nc.tensor.matmul(ps[:, 256 * j:256 * (j + 1)],
                                             W28[:, mp, :, 128 * c:128 * (c + 1)],
                                             h18[mp], start=(mp == 0), stop=(mp == 7),
                                             perf_mode=DR)
                    for j in range(2):
                        c = 2 * cc + j
                        with ALP(reason="mlp out bf16"):
                            nc.vector.tensor_scalar(
                                out=h2T[c], in0=ps[:, 256 * j:256 * (j + 1)],
                                scalar1=float(1.0 / WSC),
                                scalar2=B2C[:, c:c + 1], op0=OP.mult, op1=OP.add)
                for t in range(2):
                    tp4 = pst2.tile([128, DIM], bf16, tag="tp", name="tp")
                    for c in range(4):
                        nc.tensor.transpose(tp4[:, 128 * c:128 * (c + 1)],
                                            h2T[c][:, 128 * t:128 * (t + 1)], IDTb)
                    yo = yop.tile([128, DIM], f32, tag=f"yo{t}", name=f"yo{t}")
                    nc.vector.tensor_add(out=yo, in0=x3[t], in1=tp4)
                    t0 = 256 * qt + 128 * t
                    nc.sync.dma_start(out=y_d[b, t0:t0 + 128, :], in_=yo)

            # -------- software-pipelined chunk-pair loop --------
            # Pair p+1's loads+LN1+QKV are emitted before pair p's MLP phase
            # so DVE/PE have queued work while ACT runs the gelu table.
            G = 4
            NGRP = NCHUNK // G
            st = {}

            def phase1(gr, ld6):
                # loads + LN1 + QKV for all chunks of group gr
                copies = []
                for ch in range(G * gr, G * gr + G):
                    xr = load(ch)
                    xnTb, qTb, kTb, vNb, cps = ln1_qkv(ch, xr, ld6)
                    st[ch] = [xr, xnTb, qTb, kTb, vNb]
                    copies += cps
                return copies

            ld6 = act_table_load(TAB_LNEXP)
            phase1(0, ld6)
            for pr in range(NGRP):
                chs = tuple(range(G * pr, G * pr + G))
                for ch in chs:
                    xr, xnTb, qTb, kTb, vNb = st[ch]
                    st[ch].append(attn(ch, qTb, kTb, vNb, ld6))
                copies = []
                for ch in chs:
                    xr, xnTb, qTb, kTb, vNb, aoT8 = st[ch]
                    x3, xn2T8, cps = proj_ln2(ch, xr, xnTb, aoT8)
                    st[ch] += [x3, xn2T8]
                    copies += cps
                if pr + 1 < NGRP:
                    copies += phase1(pr + 1, ld6)
                ld10 = act_table_load(TAB_GELU, after=copies)
                for ch in chs:
                    h18, last_gelu = mlp1(ch, st[ch][7], ld10)
                    st[ch].append(h18)
                for ch in chs:
                    x3, h18 = st[ch][6], st[ch][8]
                    mlp2_store(ch, x3, h18)
                    del st[ch]
                ld6 = act_table_load(TAB_LNEXP, after=(last_gelu,))

    nc.compile()
    return nc


def _host_consts(rel_table):
    """BIASS [8,128,128] bf16 (n,m orientation, block-diag NEG, masked-q rows
    zeroed) and QMb keep-mask [128, CHUNK] bf16."""
    idx = _rel_index(WS).reshape(-1)
    bias = rel_table.reshape(-1, NH)[idx].reshape(N, NH, N)  # [n, h, m]
    qmask = _shift_mask(WS, SHIFT)
    keep = (~qmask).astype(np.float32)
    biass = np.full((NH, 128, 128), NEG, np.float32)
    for h in range(NH):
        bnm = bias[:, h, :] * keep[:, None]       # [n, m], masked-q rows -> 0
        biass[h, :64, :64] = bnm
        biass[h, 64:, 64:] = bnm
    qm = np.tile(keep, CHUNK // N)[None, :].repeat(128, 0)
    return biass.astype(ml_dtypes.bfloat16), qm.astype(ml_dtypes.bfloat16)


def kernel(**inputs):
    from concourse.bass_utils import run_bass_kernel_spmd

    f32 = np.float32
    x = np.asarray(inputs["x"], f32)
    g1 = np.asarray(inputs["ln1_g"], f32)
    bl1 = np.asarray(inputs["ln1_b"], f32)
    g2 = np.asarray(inputs["ln2_g"], f32)
    bl2 = np.asarray(inputs["ln2_b"], f32)
    wq = np.asarray(inputs["wq"], f32)
    wk = np.asarray(inputs["wk"], f32)
    wv = np.asarray(inputs["wv"], f32)
    wp = np.asarray(inputs["wp"], f32)
    bq = np.asarray(inputs["bq"], f32)
    bk = np.asarray(inputs["bk"], f32)
    bv = np.asarray(inputs["bv"], f32)
    bp = np.asarray(inputs["bp"], f32)
    w1 = np.asarray(inputs["mlp_w1"], f32)
    b1 = np.asarray(inputs["mlp_b1"], f32)
    w2 = np.asarray(inputs["mlp_w2"], f32)
    b2 = np.asarray(inputs["mlp_b2"], f32)

    # fold LN1 gamma into QKV weights; beta into their biases
    wq_e = wq * g1[None, :]
    wk_e = wk * g1[None, :]
    wv_e = wv * g1[None, :]
    bq_e = bq + wq_e @ bl1
    bk_e = bk + wk_e @ bl1
    bv_e = bv + wv_e @ bl1
    # proj bias absorbs bv (attn rows sum to 1) -> blp rides the residual
    bp3 = bp + wp @ bv_e
    blp = bl1 + bp3
    # fold LN2 gamma/beta into MLP1
    w1_e = w1 * g2[None, :]
    b1_e = b1 + w1_e @ bl2

    biass, qm = _host_consts(np.asarray(inputs["rel_table"], f32))
    sgw = _win_order_sigmoid_gate(np.asarray(inputs["gate"], f32))
    b1g = _col_tiles(b1_e, 16)
    b1_pair_eq = bool(np.allclose(b1g[:, 0::2], b1g[:, 1::2]))

    common = {
        "wq8": _pack_pairs(np.ascontiguousarray(wq_e.T)),
        "wk8": _pack_pairs(np.ascontiguousarray(wk_e.T)),
        "wv8": _pack_pairs(np.ascontiguousarray(wv_e.T)),
        "wp8": _pack_pairs(np.ascontiguousarray(wp.T)),
        "w18": _pack_pairs(np.ascontiguousarray(w1_e.T)),
        "w28": _pack_pairs(np.ascontiguousarray(w2.T)).reshape(128, 8, 2, DIM),
        "biass": biass, "qm": qm,
        "idt": np.eye(128, dtype=ml_dtypes.bfloat16),
        "sgw": sgw,
        "g1c": _col_tiles(g1, 4),
        "blp": _col_tiles(blp, 4),
        "bq64": _col_tiles(WSC * bq_e, 4),
        "bks": _col_tiles(SCALE * bk_e / WSC, 4),
        "b1g": b1g,
        "b2c": _col_tiles(b2, 4),
    }

    if "prog" not in _prog_cache:
        _prog_cache["prog"] = _build_program(b1_pair_eq)
    nc = _prog_cache["prog"]

    perm = _perm_idx()
    xw = x.reshape(B_TOTAL, TOK_IMG, DIM)[:, perm, :]
    xw8 = xw.astype(ml_dtypes.bfloat16)
    in_maps = []
    for c in range(NCORES):
        m = dict(common)
        m["x"] = np.ascontiguousarray(xw8[c * B_LOC:(c + 1) * B_LOC])
        in_maps.append(m)
    res = run_bass_kernel_spmd(nc, in_maps, core_ids=list(range(NCORES)))
    yw = np.concatenate([res.results[c]["y"] for c in range(NCORES)], axis=0)
    out = np.empty((B_TOTAL, TOK_IMG, DIM), np.float32)
    out[:, perm, :] = yw
    return out.reshape(B_TOTAL, 1, HRES, WRES, DIM).astype(np.float32)
